# revision 1
# baseline (speedup 1.0000x reference)
"""OIM loss with circular queue — Trainium2 Bass kernel (8 NeuronCores).

Strategy
--------
The output is a scalar:  loss = mean_b [ logsumexp_{q in good}(30*cos(x_b, e_q))
                                         - 30*cos(x_b, e_{xe_b}) ]
where e is the circular queue after the (sequential, data-dependent) update.

The queue update only *moves integer labels around* plus writes U=256
normalized per-pid mean embeddings into a contiguous window of slots.  All the
integer bookkeeping (which slots are invalidated, which slot each batch row
targets) is done on the host; every FLOP-heavy part runs on the 8 cores:

  - per-pid masked means  (one-hot mask matmul,  [U,B]x[B,D])
  - row normalization of inputs and means
  - the big logits matmul [B,Q//8,D] per core (float32r, full PE rate)
    fused with exp (ACT: exp(30*s - M), M a safe upper bound of the row max)
    and the masked row-sum (DVE tensor_tensor_reduce with the `good` mask)
  - target cosines via a small [B,U] matmul + one-hot gather

Sharding: emb_cq is sharded over Q (2048 rows/core, tensor-parallel); the
batch-side preprocessing is replicated (it is ~2% of the FLOPs).  Each core
returns partial sums of exp(logit - M) over its Q-shard plus the target
cosines; the host adds the 8 partials (the "allreduce"), takes log and means.
"""

import os
import sys

import numpy as np

for _p in ("/opt/trn_rl_repo", "/root/.axon_site/_ro/trn_rl_repo"):
    if os.path.isdir(_p) and _p not in sys.path:
        sys.path.insert(0, _p)

B, D, Q, U = 4096, 512, 16384, 256
N_CORES = 8
QS = Q // N_CORES          # queue rows per core
OIM_SCALAR = 30.0
IGNORE = -1
MT = B // 128              # 32 b-tiles
QT = QS // 128             # 16 q-tiles per core
KD = D // 128              # 4 contraction chunks
NQ = QS // 512             # 4 matmul n-chunks per core
UT = U // 128              # 2 u-tiles

_PROG_CACHE = {}


def _build_program(M: float, work_bufs=4, psm_bufs=2, kd_outer=False, pst_bufs=4, small_bufs=6, exp_bufs=6, tl_bufs=4):
    """Emit + schedule + compile the (SPMD, identical on all cores) program."""
    import concourse.bacc as bacc
    import concourse.bass as bass
    import concourse.tile as tile
    from concourse import mybir
    from concourse.masks import make_identity

    f32 = mybir.dt.float32
    f32r = mybir.dt.float32r
    AF = mybir.ActivationFunctionType
    OP = mybir.AluOpType

    nc = bacc.Bacc("TRN2", target_bir_lowering=False, debug=False,
                   num_devices=N_CORES)

    x_d = nc.dram_tensor("x", [B, D], f32, kind="ExternalInput").ap()
    emb_d = nc.dram_tensor("emb", [QS, D], f32, kind="ExternalInput").ap()
    labf_d = nc.dram_tensor("labf", [128, MT], f32, kind="ExternalInput").ap()
    uniqf_d = nc.dram_tensor("uniqf", [128, U], f32, kind="ExternalInput").ap()
    cnts_d = nc.dram_tensor("cnts", [128, UT], f32, kind="ExternalInput").ap()
    widx_d = nc.dram_tensor("widx", [128, MT], f32, kind="ExternalInput").ap()
    iota_d = nc.dram_tensor("iota", [128, U], f32, kind="ExternalInput").ap()
    gkeep_d = nc.dram_tensor("gkeep", [128, QS], f32, kind="ExternalInput").ap()
    wkeep_d = nc.dram_tensor("wkeep", [128, QT], f32, kind="ExternalInput").ap()
    oht_d = nc.dram_tensor("oht", [128, UT, QS], f32, kind="ExternalInput").ap()
    sume_d = nc.dram_tensor("sume", [128, MT], f32, kind="ExternalOutput").ap()
    tco_d = nc.dram_tensor("tco", [128, MT], f32, kind="ExternalOutput").ap()
    tick_d = nc.dram_tensor("tick", [128, 4], f32, kind="ExternalInput").ap()
    tock_d = nc.dram_tensor("tock", [128, 4], f32, kind="ExternalOutput").ap()

    with tile.TileContext(nc) as tc:
        with (
            tc.tile_pool(name="singles", bufs=1) as singles,
            tc.tile_pool(name="work", bufs=work_bufs) as work,
            tc.tile_pool(name="small", bufs=small_bufs) as small,
            tc.tile_pool(name="psum_t", bufs=pst_bufs, space="PSUM") as psum_t,
        ):
            # ---------------- constants / small inputs ----------------
            ident = singles.tile([128, 128], f32)
            make_identity(nc, ident)

            labs = singles.tile([128, MT], f32)
            nc.sync.dma_start(out=labs, in_=labf_d)
            widx = singles.tile([128, MT], f32)
            nc.sync.dma_start(out=widx, in_=widx_d)
            wkp = singles.tile([128, QT], f32)
            nc.sync.dma_start(out=wkp, in_=wkeep_d)
            cnts = singles.tile([128, UT], f32)
            nc.sync.dma_start(out=cnts, in_=cnts_d)
            uniqb = singles.tile([128, U], f32)
            nc.sync.dma_start(out=uniqb, in_=uniqf_d)
            iotab = singles.tile([128, U], f32)
            nc.sync.dma_start(out=iotab, in_=iota_d)
            keepg = singles.tile([128, QS], f32)
            nc.sync.dma_start(out=keepg, in_=gkeep_d)
            oht = singles.tile([128, UT, QS], f32r)
            nc.sync.dma_start(out=oht, in_=oht_d.bitcast(f32r))

            rcnt = singles.tile([128, UT], f32)
            nc.vector.reciprocal(rcnt, cnts)
            biasM = singles.tile([128, 1], f32)
            nc.vector.memset(biasM, -M)

            # resident big tensors
            xn_all = singles.tile([128, MT, D], f32)     # normalized inputs (b-major)
            embT = singles.tile([128, KD, QS], f32r)     # blended emb, d-major
            uembT = singles.tile([128, KD, U], f32r)     # uniq means, d-major
            uemb_n = singles.tile([128, UT, D], f32r)    # uniq means, u-major
            ssb = singles.tile([128, MT], f32)           # sum-exp out collector
            tsb = singles.tile([128, MT], f32)           # target-cos out collector

            # ---------------- phase 1+2: masked means + normalize ----------
            with tc.tile_pool(name="psum_u", bufs=1, space="PSUM") as psum_u:
                ps_u = [psum_u.tile([128, D], f32, tag=f"uniq{mu}",
                                    name=f"ps_u{mu}") for mu in range(UT)]
                for i in range(MT):
                    x_raw = work.tile([128, D], f32r, tag="x_raw")
                    nc.sync.dma_start(out=x_raw,
                                      in_=x_d[i * 128:(i + 1) * 128, :].bitcast(f32r))
                    x_f = x_raw.bitcast(f32)

                    # mask[b, u] = (uniq[u] == labels[b])
                    mt_ = work.tile([128, U], f32r, tag="maskr")
                    nc.vector.tensor_scalar(out=mt_, in0=uniqb,
                                            scalar1=labs[:, i:i + 1], scalar2=None,
                                            op0=OP.is_equal)
                    for mu in range(UT):
                        nc.tensor.matmul(ps_u[mu],
                                         mt_[:, mu * 128:(mu + 1) * 128],
                                         x_raw, start=(i == 0),
                                         stop=(i == MT - 1))

                    # row-normalize x
                    sq = work.tile([128, D], f32, tag="sq")
                    ssq = small.tile([128, 1], f32, tag="ssq")
                    nc.vector.scalar_tensor_tensor(out=sq, in0=x_f, scalar=1.0,
                                                   in1=x_f, op0=OP.mult,
                                                   op1=OP.mult, accum_out=ssq)
                    nrm = small.tile([128, 1], f32, tag="nrm")
                    nc.scalar.activation(out=nrm, in_=ssq, func=AF.Sqrt)
                    nc.vector.tensor_scalar_max(out=nrm, in0=nrm, scalar1=1e-12)
                    rin = small.tile([128, 1], f32, tag="rin")
                    nc.vector.reciprocal(rin, nrm)
                    nc.vector.tensor_scalar_mul(out=xn_all[:, i, :], in0=x_f,
                                                scalar1=rin)

                # finish uniq means: mean, normalize, transpose to d-major
                for mu in range(UT):
                    ue = uemb_n[:, mu, :]
                    nc.vector.tensor_scalar_mul(out=ue, in0=ps_u[mu],
                                                scalar1=rcnt[:, mu:mu + 1])
                    sq2 = work.tile([128, D], f32, tag="sq")
                    ssq2 = small.tile([128, 1], f32, tag="ssq")
                    ue_f = ue.bitcast(f32)
                    nc.vector.scalar_tensor_tensor(out=sq2, in0=ue_f, scalar=1.0,
                                                   in1=ue_f, op0=OP.mult,
                                                   op1=OP.mult, accum_out=ssq2)
                    nrm2 = small.tile([128, 1], f32, tag="nrm")
                    nc.scalar.activation(out=nrm2, in_=ssq2, func=AF.Sqrt)
                    nc.vector.tensor_scalar_max(out=nrm2, in0=nrm2, scalar1=1e-12)
                    rin2 = small.tile([128, 1], f32, tag="rin")
                    nc.vector.reciprocal(rin2, nrm2)
                    nc.vector.tensor_scalar_mul(out=ue, in0=ue_f, scalar1=rin2)
                    for kd in range(KD):
                        pst = psum_t.tile([128, 128], f32, tag="pst")
                        nc.tensor.transpose(
                            pst,
                            uemb_n[:, mu, kd * 128:(kd + 1) * 128].bitcast(f32),
                            ident)
                        nc.scalar.copy(out=uembT[:, kd, mu * 128:(mu + 1) * 128],
                                       in_=pst)

            # ---------------- phase 3: blend queue window + transpose ------
            with tc.tile_pool(name="psum_b", bufs=2, space="PSUM") as psum_b:
                for t in range(QT):
                    e_raw = work.tile([128, D], f32, tag="e_raw")
                    nc.sync.dma_start(out=e_raw,
                                      in_=emb_d[t * 128:(t + 1) * 128, :])
                    eff = work.tile([128, D], f32, tag="eff")
                    # zero the window rows ...
                    nc.vector.tensor_scalar_mul(out=eff, in0=e_raw,
                                                scalar1=wkp[:, t:t + 1])
                    # ... and add one-hot @ uniq_means
                    psb = psum_b.tile([128, D], f32, tag="psb")
                    for ku in range(UT):
                        nc.tensor.matmul(psb,
                                         oht[:, ku, t * 128:(t + 1) * 128],
                                         uemb_n[:, ku, :],
                                         start=(ku == 0), stop=(ku == UT - 1))
                    nc.vector.tensor_add(out=eff, in0=eff, in1=psb)
                    for kd in range(KD):
                        pst = psum_t.tile([128, 128], f32, tag="pst")
                        nc.tensor.transpose(pst, eff[:, kd * 128:(kd + 1) * 128],
                                            ident)
                        nc.scalar.copy(out=embT[:, kd, t * 128:(t + 1) * 128],
                                       in_=pst)

            # ---------------- phase 4: logits + fused LSE ----------------
            with (
                tc.tile_pool(name="psum_s", bufs=2, space="PSUM") as psum_s,
                tc.tile_pool(name="psum_m", bufs=psm_bufs, space="PSUM") as psum_m,
            ):
                for m in range(MT):
                    tl = work.tile([128, D], f32r, tag="lhsT", bufs=tl_bufs)
                    for kd in range(KD):
                        pst = psum_t.tile([128, 128], f32, tag="pst")
                        nc.tensor.transpose(
                            pst, xn_all[:, m, kd * 128:(kd + 1) * 128], ident)
                        nc.scalar.copy(out=tl[:, kd * 128:(kd + 1) * 128], in_=pst)

                    # target cosines: S2[b, u] then one-hot gather along u
                    pss = psum_s.tile([128, U], f32, tag="pss")
                    for kd in range(KD):
                        nc.tensor.matmul(pss, tl[:, kd * 128:(kd + 1) * 128],
                                         uembT[:, kd, :],
                                         start=(kd == 0), stop=(kd == KD - 1))
                    scr_u = work.tile([128, U], f32, tag="mask")
                    nc.vector.scalar_tensor_tensor(out=scr_u, in0=iotab,
                                                   scalar=widx[:, m:m + 1],
                                                   in1=pss,
                                                   op0=OP.is_equal, op1=OP.mult,
                                                   accum_out=tsb[:, m:m + 1])

                    # big matmul over this core's Q-shard, fused exp+masked sum
                    acc4 = small.tile([128, NQ], f32, tag="acc4")
                    if kd_outer:
                        psms = [psum_m.tile([128, 512], f32, tag=f"psm{n}",
                                            name=f"psm_{m}_{n}") for n in range(NQ)]
                        for kd in range(KD):
                            for n in range(NQ):
                                nc.tensor.matmul(
                                    psms[n], tl[:, kd * 128:(kd + 1) * 128],
                                    embT[:, kd, n * 512:(n + 1) * 512],
                                    start=(kd == 0), stop=(kd == KD - 1))
                        for n in range(NQ):
                            expt = work.tile([128, 512], f32, tag="expt", bufs=exp_bufs)
                            nc.scalar.activation(out=expt, in_=psms[n], func=AF.Exp,
                                                 bias=biasM, scale=OIM_SCALAR)
                            scr = work.tile([128, 512], f32, tag="scr", bufs=exp_bufs)
                            nc.vector.scalar_tensor_tensor(
                                out=scr, in0=expt, scalar=1.0,
                                in1=keepg[:, n * 512:(n + 1) * 512],
                                op0=OP.mult, op1=OP.mult,
                                accum_out=acc4[:, n:n + 1])
                    else:
                        for n in range(NQ):
                            psm = psum_m.tile([128, 512], f32, tag="psm")
                            for kd in range(KD):
                                nc.tensor.matmul(
                                    psm, tl[:, kd * 128:(kd + 1) * 128],
                                    embT[:, kd, n * 512:(n + 1) * 512],
                                    start=(kd == 0), stop=(kd == KD - 1))
                            expt = work.tile([128, 512], f32, tag="expt", bufs=exp_bufs)
                            nc.scalar.activation(out=expt, in_=psm, func=AF.Exp,
                                                 bias=biasM, scale=OIM_SCALAR)
                            scr = work.tile([128, 512], f32, tag="scr", bufs=exp_bufs)
                            nc.vector.scalar_tensor_tensor(
                                out=scr, in0=expt, scalar=1.0,
                                in1=keepg[:, n * 512:(n + 1) * 512],
                                op0=OP.mult, op1=OP.mult,
                                accum_out=acc4[:, n:n + 1])
                    nc.vector.reduce_sum(out=ssb[:, m:m + 1], in_=acc4,
                                         axis=mybir.AxisListType.X)

            nc.sync.dma_start(out=sume_d, in_=ssb)
            nc.sync.dma_start(out=tco_d, in_=tsb)
            tickt = singles.tile([128, 4], f32)
            nc.sync.dma_start(out=tickt, in_=tick_d)
            nc.sync.dma_start(out=tock_d, in_=tickt)

    nc.compile()
    return nc


def _host_bookkeeping(labels, label_cq, header_cq):
    """Mirror the reference's integer-only queue-update semantics."""
    labels = np.asarray(labels).astype(np.int64)
    lab = np.asarray(label_cq).astype(np.int64).copy()
    h0 = int(np.asarray(header_cq))

    # jnp.unique(labels, size=U): sorted unique, padded with the minimum
    uq = np.unique(labels)
    if uq.size < U:
        uniq = np.concatenate([uq, np.full(U - uq.size, uq.min(), np.int64)])
    else:
        uniq = uq[:U]
    cnts = np.array([(labels == v).sum() for v in uniq], np.int64)

    emb_src = np.full(Q, -1, np.int64)   # >=0: row u of uniq means; -1: original
    h = h0 % Q
    for u in range(U):
        y = uniq[u]
        m = lab == y
        i = int(np.argmax(m)) if m.any() else 0
        inval = bool(m.any()) and (i != h)
        emb_src[h] = u
        lab[h] = y
        if inval:
            lab[i] = IGNORE
        h = (h + 1) % Q

    good = lab != IGNORE
    goodidx = np.flatnonzero(good)
    gl = lab[goodidx]
    vals, first = np.unique(gl, return_index=True)
    pos = np.searchsorted(vals, labels)
    assert np.all(vals[np.clip(pos, 0, vals.size - 1)] == labels), \
        "batch label missing from queue"
    xe = goodidx[first[pos]]
    return uniq, cnts, emb_src, good, xe


def _prepare(inputs, labels, emb_cq, label_cq, header_cq):
    """Host bookkeeping -> (M, per-core input maps, extra-target indices, xe)."""
    inputs = np.ascontiguousarray(np.asarray(inputs, np.float32))
    emb_cq = np.ascontiguousarray(np.asarray(emb_cq, np.float32))

    uniq, cnts, emb_src, good, xe = _host_bookkeeping(labels, label_cq, header_cq)

    # safe upper bound for any logit: 30 * max row norm (uniq means have norm 1)
    max_nrm = float(np.sqrt((emb_cq.astype(np.float64) ** 2).sum(axis=1).max()))
    M = OIM_SCALAR * max(1.0, max_nrm) * 1.0000001

    w_idx = emb_src[xe].astype(np.float64)        # -1 for non-window targets
    extra = np.flatnonzero(w_idx < 0)             # handled on host (rare/none)

    def pmajor(v, cols):
        return np.ascontiguousarray(
            np.asarray(v, np.float32).reshape(cols, 128).T)

    base = {
        "x": inputs,
        "tick": np.zeros((128, 4), np.float32),
        "labf": pmajor(np.asarray(labels, np.float64), MT),
        "uniqf": np.ascontiguousarray(
            np.broadcast_to(uniq.astype(np.float32), (128, U))),
        "cnts": pmajor(cnts, UT),
        "widx": pmajor(w_idx, MT),
        "iota": np.ascontiguousarray(
            np.broadcast_to(np.arange(U, dtype=np.float32), (128, U))),
    }
    in_maps = []
    for c in range(N_CORES):
        sl = slice(c * QS, (c + 1) * QS)
        src = emb_src[sl]
        ohtT = np.zeros((U, QS), np.float32)
        j = np.flatnonzero(src >= 0)
        ohtT[src[j], j] = 1.0
        in_maps.append({
            **base,
            "emb": np.ascontiguousarray(emb_cq[sl]),
            "gkeep": np.ascontiguousarray(
                np.broadcast_to(good[sl].astype(np.float32), (128, QS))),
            "wkeep": pmajor((src < 0).astype(np.float32), QT),
            "oht": np.ascontiguousarray(
                ohtT.reshape(UT, 128, QS).transpose(1, 0, 2)),
        })
    return M, in_maps, extra, xe


def _combine(res_list, M, extra, xe, inputs, emb_cq):
    """Unshard / combine per-core partials into the scalar loss."""
    S = np.zeros(B, np.float64)
    for r in res_list:
        S += r["sume"].astype(np.float64).T.reshape(B)
    t_cos = res_list[0]["tco"].astype(np.float64).T.reshape(B)

    if extra.size:  # targets pointing at original (non-window) queue rows
        xb = np.asarray(inputs, np.float64)[extra]
        xb /= np.maximum(np.linalg.norm(xb, axis=1, keepdims=True), 1e-12)
        eb = np.asarray(emb_cq, np.float64)[xe[extra]]
        t_cos[extra] = (xb * eb).sum(axis=1)

    loss = np.mean(M + np.log(S) - OIM_SCALAR * t_cos)
    return np.array(loss, dtype=np.float32)


def kernel(inputs, labels, emb_cq, label_cq, age_cq, header_cq):
    from concourse.bass_utils import run_bass_kernel_spmd

    M, in_maps, extra, xe = _prepare(inputs, labels, emb_cq, label_cq, header_cq)

    key = round(M, 9)
    if key not in _PROG_CACHE:
        _PROG_CACHE[key] = _build_program(M)
    nc = _PROG_CACHE[key]

    res = run_bass_kernel_spmd(nc, in_maps, core_ids=list(range(N_CORES)))
    return _combine(res.results, M, extra, xe, inputs, emb_cq)



# revision 26
# speedup vs baseline: 3.0824x; 3.0824x over previous
"""OIM loss with circular queue — Trainium2 Bass kernel (8 NeuronCores).

loss = mean_b [ M + log(sum_{q good} exp(30*cos(x_b,e_q) - M)) - 30*cos(x_b,e_{xe_b}) ]

where e is the circular queue after the (sequential, data-dependent) update.
The update writes U=256 normalized per-pid mean embeddings into a contiguous
window of slots and invalidates stale slots; the integer bookkeeping runs on
the host, the FLOPs on the 8 cores.

Sharding (tensor-parallel over Q; we own the shard assignment):
  * each core owns 32 of the 256 window slots (placed at columns 0..31 of its
    shard) plus up to 2016 good non-window queue slots (zero-padded);
    invalidated/bad slots are never shipped, so no masking is needed — the
    exp row-sum comes straight from the ACT accumulator.
  * the host ships emb pre-transposed (d-major) fp8-quantized in DoubleRow
    layout, and x twice: b-major bf16 (masked means) and d-major fp8
    DoubleRow tiles of the row-normalized x (per-row scaling folded into the
    fp8 quantization), so the logits matmul emits cosines directly.

Per-core device program:
  phase A (streamed, 8 batches of 4 b-tiles): masked-sum matmuls (bf16)
    accumulate in PSUM.
  finalize: normalize the 32 per-pid mean rows (norm of the raw sum — the
    count cancels; rsqrt via Newton on DVE, keeping ACT exp-only),
    PE-transpose into the window columns of the emb tile.
  phase C (streamed over 32 b-tiles): 8 fp8 DoubleRow matmuls fill a
    [128,2048] PSUM tile with cosines; one Exp activation (scale=30,
    bias=-M) yields the row sum via the ACT accumulator; a small DVE
    gather extracts the target cosine from the 32 window columns.
Host: S_b = sum_c sume_c - n_zero*e^-M;  loss = mean(M + log S_b - t30_b).
"""

import os
import sys

import numpy as np

for _p in ("/opt/trn_rl_repo", "/root/.axon_site/_ro/trn_rl_repo"):
    if os.path.isdir(_p) and _p not in sys.path:
        sys.path.insert(0, _p)

B, D, Q, U = 4096, 512, 16384, 256
N_CORES = 8
UC = U // N_CORES           # 32 window slots / uniq pids per core
NW = 2016                   # non-window columns per core (zero-padded)
QSC = UC + NW               # 2048 queue columns per core
MT = B // 128               # 32 b-tiles
MB = 4                      # b-tiles per DMA batch
XG = 1024                   # gathered rows per core for the masked means
GT = XG // 128              # 8 gather tiles
OIM_SCALAR = 30.0
M_BIAS = 30.0               # logits are <= 30 (both sides unit-norm)
IGNORE = -1
RSQRT_MAGIC = 0x5F3759DF

_PROG_CACHE = {}

# psum/rhs column chunks — each is one full 2KB PSUM bank and ONE matmul
# accumulation group (PSUM start/stop semantics are bank-granular).
_CHUNKS = [(0, 512), (512, 1024), (1024, 1536), (1536, 2048)]


def _build_program(variant="fp8"):
    import concourse.bacc as bacc
    import concourse.tile as tile
    from concourse import mybir
    from concourse.masks import make_identity

    f32 = mybir.dt.float32
    f32r = mybir.dt.float32r
    i32 = mybir.dt.int32
    bf16 = mybir.dt.bfloat16
    fp8 = mybir.dt.float8e4
    AF = mybir.ActivationFunctionType
    OP = mybir.AluOpType
    DR = mybir.MatmulPerfMode.DoubleRow

    fp8_mode = variant == "fp8"
    e_dt = fp8 if fp8_mode else f32        # storage dtype of x^T / emb^T
    KD = 2 if fp8_mode else 4              # matmul contraction chunks
    SUB = D // (128 * KD)                  # 2 (DoubleRow pair) or 1

    def as_mm(ap):                         # matmul-operand view
        return ap if fp8_mode else ap.bitcast(f32r)

    nc = bacc.Bacc("TRN2", target_bir_lowering=False, debug=False,
                   num_devices=N_CORES)

    xg_d = nc.dram_tensor("xg", [XG, D], bf16, kind="ExternalInput").ap()
    xt_d = nc.dram_tensor("xt", [B, D], e_dt, kind="ExternalInput").ap()
    emb_d = nc.dram_tensor("emb", [128, (D // 128) * NW], e_dt,
                           kind="ExternalInput").ap()
    meta_d = nc.dram_tensor("meta", [128, GT + UC + MT + UC], f32,
                            kind="ExternalInput").ap()
    sume_d = nc.dram_tensor("sume", [128, MT], f32, kind="ExternalOutput").ap()
    tco_d = nc.dram_tensor("tco", [128, MT], f32, kind="ExternalOutput").ap()

    with tile.TileContext(nc) as tc:
        with (
            tc.tile_pool(name="singles", bufs=1) as singles,
            tc.tile_pool(name="xwork", bufs=3) as xwork,
            tc.tile_pool(name="twork", bufs=3) as twork,
            tc.tile_pool(name="mwork", bufs=4) as mwork,
            tc.tile_pool(name="ework", bufs=2) as ework,
        ):
            # ---------- small resident inputs, one DMA ----------
            meta = singles.tile([128, GT + UC + MT + UC], f32)
            nc.sync.dma_start(out=meta, in_=meta_d)
            uniqc = meta[:, GT:GT + UC]
            iotac = meta[:, GT + UC + MT:]
            # per-column scalar operands must be slices of real tiles (the
            # dependency tracker does not follow slice-of-slice reads)
            labg = singles.tile([128, GT], f32)
            nc.vector.tensor_copy(out=labg, in_=meta[:, 0:GT])
            widx = singles.tile([128, MT], f32)
            nc.vector.tensor_copy(out=widx, in_=meta[:, GT + UC:GT + UC + MT])

            biasM = singles.tile([128, 1], f32)
            nc.vector.memset(biasM, -M_BIAS)
            # preload the Exp activation table while DMAs stream
            junk1 = singles.tile([128, 1], f32)
            nc.scalar.activation(out=junk1, in_=biasM, func=AF.Exp)

            ident = singles.tile([128, 128], f32)
            make_identity(nc, ident)

            sacc = singles.tile([128, MT], f32)     # sum-exp out
            tsb = singles.tile([128, MT], f32)      # target cosine
            tco30 = singles.tile([128, MT], f32)    # 30*cos target out
            # queue columns: 0..31 window (written on-device), 32.. from DMA
            embt = singles.tile([128, KD, SUB, QSC], e_dt)

            # ---------- phase A: masked sums ----------
            with (
                tc.tile_pool(name="psA", bufs=1, space="PSUM") as psA,
                tc.tile_pool(name="psT", bufs=2, space="PSUM") as psT,
            ):
                ps_u = psA.tile([UC, D], f32, name="ps_u")
                for mb in range(GT // MB):
                    xg4 = xwork.tile([128, MB, D], bf16, tag="xg")
                    nc.sync.dma_start(
                        out=xg4,
                        in_=xg_d[mb * MB * 128:(mb + 1) * MB * 128, :]
                        .rearrange("(j p) d -> p j d", j=MB))
                    for j in range(MB):
                        m = mb * MB + j
                        mask = mwork.tile([128, UC], bf16, tag="mask")
                        nc.vector.tensor_scalar(out=mask, in0=uniqc,
                                                scalar1=labg[:, m:m + 1],
                                                scalar2=None, op0=OP.is_equal)
                        nc.tensor.matmul(ps_u, mask, xg4[:, j],
                                         start=(m == 0), stop=(m == GT - 1))

                # emb shard lands right after the xb stream, before phase C
                nc.sync.dma_start(
                    out=embt[:, :, :, UC:],
                    in_=emb_d.rearrange("p (a b c) -> p a b c", a=KD, b=SUB))

                # ---------- finalize: normalized means -> window cols ------
                uembS = singles.tile([UC, D], f32)
                nc.vector.tensor_copy(out=uembS, in_=ps_u)
                scrU = singles.tile([UC, D], f32)
                ssqu = singles.tile([UC, 1], f32)
                nc.vector.scalar_tensor_tensor(
                    out=scrU, in0=uembS, scalar=1.0, in1=uembS,
                    op0=OP.mult, op1=OP.mult, accum_out=ssqu)
                # rinvu = rsqrt(ssqu) via bit-trick + 2 Newton steps (DVE-only
                # so the ACT engine never swaps away from the Exp table)
                yi = singles.tile([UC, 1], i32)
                nc.vector.tensor_scalar(
                    out=yi, in0=ssqu.bitcast(i32), scalar1=1,
                    scalar2=None, op0=OP.arith_shift_right)
                nc.vector.tensor_scalar(
                    out=yi, in0=yi, scalar1=-1, scalar2=RSQRT_MAGIC,
                    op0=OP.mult, op1=OP.add)
                y = yi.bitcast(f32)
                t0 = singles.tile([UC, 1], f32)
                for _ in range(2):
                    nc.vector.tensor_tensor(out=t0, in0=y, in1=y, op=OP.mult)
                    nc.vector.tensor_tensor(out=t0, in0=t0, in1=ssqu,
                                            op=OP.mult)
                    nc.vector.tensor_scalar(out=t0, in0=t0, scalar1=-0.5,
                                            scalar2=1.5, op0=OP.mult,
                                            op1=OP.add)
                    nc.vector.tensor_tensor(out=y, in0=y, in1=t0, op=OP.mult)
                uembn = singles.tile([UC, D], f32)
                nc.vector.tensor_scalar_mul(out=uembn, in0=uembS, scalar1=y)
                for kd in range(4):
                    pst = psT.tile([128, UC], f32, tag="pst")
                    nc.tensor.transpose(pst,
                                        uembn[:, kd * 128:(kd + 1) * 128],
                                        ident[0:UC, 0:UC])
                    if fp8_mode:
                        dst = embt[:, kd // 2, kd % 2, 0:UC]
                    else:
                        dst = embt[:, kd, 0, 0:UC]
                    nc.vector.tensor_copy(out=dst, in_=pst)

            # ---------- phase C: cosines + fused exp/sum + target gather ---
            with tc.tile_pool(name="psC", bufs=2, space="PSUM") as psC:
                for mb in range(MT // MB):
                    tl4 = twork.tile([128, MB, D], e_dt, tag="tl")
                    nc.sync.dma_start(
                        out=tl4,
                        in_=xt_d[mb * MB * 128:(mb + 1) * MB * 128, :]
                        .rearrange("(j p) d -> p j d", j=MB))
                    for j in range(MB):
                        m = mb * MB + j
                        tlm = tl4[:, j].rearrange("p (a b c) -> p a b c",
                                                  a=KD, b=SUB)
                        psm = psC.tile([128, QSC], f32, tag="psm")
                        for kd in range(KD):
                            lhs = as_mm(tlm[:, kd] if fp8_mode
                                        else tlm[:, kd, 0])
                            pm = DR if fp8_mode else None
                            for (p0, p1) in _CHUNKS:
                                rC = as_mm(embt[:, kd, :, p0:p1] if fp8_mode
                                           else embt[:, kd, 0, p0:p1])
                                nc.tensor.matmul(psm[:, p0:p1], lhs, rC,
                                                 start=(kd == 0),
                                                 stop=(kd == KD - 1),
                                                 perf_mode=pm)
                        g32 = mwork.tile([128, UC], f32, tag="g32")
                        nc.vector.scalar_tensor_tensor(
                            out=g32, in0=iotac, scalar=widx[:, m:m + 1],
                            in1=psm[:, 0:UC], op0=OP.is_equal, op1=OP.mult,
                            accum_out=tsb[:, m:m + 1])
                        expt = ework.tile([128, QSC], bf16, tag="expt")
                        nc.scalar.activation(out=expt, in_=psm, func=AF.Exp,
                                             bias=biasM, scale=OIM_SCALAR,
                                             accum_out=sacc[:, m:m + 1])

            nc.vector.tensor_scalar_mul(out=tco30, in0=tsb,
                                        scalar1=OIM_SCALAR)
            nc.sync.dma_start(out=sume_d, in_=sacc)
            nc.sync.dma_start(out=tco_d, in_=tco30)

    nc.compile()
    return nc


def _host_bookkeeping(labels, label_cq, header_cq):
    """Mirror the reference's integer-only queue-update semantics."""
    labels = np.asarray(labels).astype(np.int64)
    lab = np.asarray(label_cq).astype(np.int64).copy()
    h0 = int(np.asarray(header_cq))

    uq = np.unique(labels)
    if uq.size < U:
        uniq = np.concatenate([uq, np.full(U - uq.size, uq.min(), np.int64)])
    else:
        uniq = uq[:U]

    emb_src = np.full(Q, -1, np.int64)   # >=0: window slot written by uniq u
    h = h0 % Q
    for u in range(U):
        y = uniq[u]
        m = lab == y
        i = int(np.argmax(m)) if m.any() else 0
        inval = bool(m.any()) and (i != h)
        emb_src[h] = u
        lab[h] = y
        if inval:
            lab[i] = IGNORE
        h = (h + 1) % Q

    good = lab != IGNORE
    goodidx = np.flatnonzero(good)
    gl = lab[goodidx]
    vals, first = np.unique(gl, return_index=True)
    pos = np.searchsorted(vals, labels)
    assert np.all(vals[np.clip(pos, 0, vals.size - 1)] == labels), \
        "batch label missing from queue"
    xe = goodidx[first[pos]]
    return uniq, emb_src, good, xe


def _pmajor(v, cols, dt):
    return np.ascontiguousarray(np.asarray(v, np.float64)
                                .reshape(cols, 128).T.astype(dt))


def _prepare(inputs, labels, emb_cq, label_cq, header_cq, variant):
    import ml_dtypes
    bf16 = ml_dtypes.bfloat16
    fp8_mode = variant == "fp8"
    e_dt = ml_dtypes.float8_e4m3 if fp8_mode else np.float32
    KD = 2 if fp8_mode else 4
    SUB = D // (128 * KD)

    x = np.ascontiguousarray(np.asarray(inputs, np.float32))
    emb_cq = np.ascontiguousarray(np.asarray(emb_cq, np.float32))

    uniq, emb_src, good, xe = _host_bookkeeping(labels, label_cq, header_cq)

    w_idx = emb_src[xe]                       # target window index, -1=extra
    extra = np.flatnonzero(w_idx < 0)

    # window slot of uniq u; invalidated duplicates become zero columns
    h0 = int(np.asarray(header_cq)) % Q
    wslot = (h0 + np.arange(U)) % Q
    u_valid = good[wslot]
    uniq_send = np.where(u_valid, uniq, -999).astype(np.float64)

    # d-major row-normalized fp8 x for the logits lhsT (the per-row 1/|x| is
    # folded into the quantization)
    xn = x / np.maximum(np.linalg.norm(x, axis=1, keepdims=True), 1e-12)
    Y = xn.astype(e_dt).reshape(MT, 128, KD, SUB, 128)
    xt = np.ascontiguousarray(Y.transpose(0, 4, 2, 3, 1).reshape(B, D))
    xbf = x.astype(bf16)
    labels_i = np.asarray(labels).astype(np.int64)

    # queue columns: good non-window slots split across cores
    nonwin = np.flatnonzero(good & (emb_src < 0))
    parts = np.array_split(nonwin, N_CORES)

    base = {"xt": xt}
    widx_pm = _pmajor(w_idx, MT, np.float32)
    in_maps = []
    n_pad_total = 0
    for c in range(N_CORES):
        cols = parts[c]
        n_pad_total += NW - cols.size
        E = np.zeros((NW, D), np.float32)
        E[: cols.size] = emb_cq[cols]
        Z = E.astype(e_dt).reshape(NW, KD, SUB, 128)
        embp = np.ascontiguousarray(
            Z.transpose(3, 1, 2, 0).reshape(128, KD * SUB * NW))
        # rows whose labels fall in this core's uniq set (masked-mean input)
        uc_vals = uniq_send[c * UC:(c + 1) * UC]
        rows = np.flatnonzero(np.isin(labels_i, uc_vals[uc_vals >= 0]))
        assert rows.size <= XG, f"core {c}: {rows.size} gathered rows > {XG}"
        xg = np.zeros((XG, D), bf16)
        xg[: rows.size] = xbf[rows]
        labgv = np.full(XG, -1.0, np.float64)
        labgv[: rows.size] = labels_i[rows]
        meta = np.concatenate([
            _pmajor(labgv, GT, np.float32),
            np.broadcast_to(uc_vals.astype(np.float32), (128, UC)),
            widx_pm,
            np.broadcast_to(np.arange(c * UC, (c + 1) * UC,
                                      dtype=np.float32), (128, UC)),
        ], axis=1)
        in_maps.append({
            **base,
            "emb": embp,
            "xg": xg,
            "meta": np.ascontiguousarray(meta),
        })
    # zero columns (padding + invalidated window slots) each add e^-M per row
    n_const = n_pad_total + int((~u_valid).sum())
    return in_maps, extra, xe, n_const, (x, emb_cq)


def _combine(res_list, extra, xe, n_const, xemb):
    x, emb_cq = xemb
    S = np.zeros(B, np.float64)
    t30 = np.zeros(B, np.float64)
    for r in res_list:
        S += r["sume"].astype(np.float64).T.reshape(B)
        t30 += r["tco"].astype(np.float64).T.reshape(B)
    S -= n_const * np.exp(-float(M_BIAS))

    if extra.size:  # targets pointing at original (non-window) queue rows
        xb = x[extra].astype(np.float64)
        xb /= np.maximum(np.linalg.norm(xb, axis=1, keepdims=True), 1e-12)
        eb = emb_cq[xe[extra]].astype(np.float64)
        t30[extra] = OIM_SCALAR * (xb * eb).sum(axis=1)

    loss = np.mean(M_BIAS + np.log(S) - t30)
    return np.array(loss, dtype=np.float32)


def kernel(inputs, labels, emb_cq, label_cq, age_cq, header_cq):
    from concourse.bass_utils import run_bass_kernel_spmd

    variant = os.environ.get("BASS_VARIANT", "fp8")
    in_maps, extra, xe, n_const, xemb = _prepare(
        inputs, labels, emb_cq, label_cq, header_cq, variant)

    if variant not in _PROG_CACHE:
        _PROG_CACHE[variant] = _build_program(variant)
    nc = _PROG_CACHE[variant]

    res = run_bass_kernel_spmd(nc, in_maps, core_ids=list(range(N_CORES)))
    return _combine(res.results, extra, xe, n_const, xemb)
